# revision 54
# baseline (speedup 1.0000x reference)
"""Trainium2 Bass kernel for nn_MultiHeadAttention (B=8, S=1024, HID=1024, NH=16).

Strategy: data-parallel over batch — core b computes the full MHA for batch
element b (B == n_cores == 8, no collectives).

The kernel is organized to keep the PE (tensor engine) queue dense end-to-end
so the HAM clock gate stays at 2.4 GHz (scattered micro-idles re-throttle it
to 1.2 GHz, doubling every matmul):

  Prologue:  xT = x.T (PE transpose, bf16)
             E  = bf16 exp(-1e9*(mask - rowmin(mask))).T   (shared by heads)
             wq/wk/wv converted to bf16 on-chip
             QT/KT for head pair 0 (d-major, bf16);  V for ALL heads (bf16,
             s-major, +ones column per head for the softmax denominator)
  Attention: flat stream over (head, ktile); per ktile:
               S[k,q] = KT_h.T @ QT_h         (bf16, PSUM, 2x 512 chunks)
               A      = exp(S * 1/32)         (ACT, bf16)
               A     *= E[ki]                 (DVE bf16 2x mode)
               CX    += V'_h[ki].T @ A        (bf16; row 64 = denom)
             CX matmuls lag the score matmuls by 2 ktiles; QT/KT projection
             matmuls for head pair j+1 are interleaved as PE filler during
             pair j so the PE never idles.
             Per-head tail (DVE/DMA/gpsimd, off critical path): evict
             unnormalized ctx, denom -> [8,128] reshape via DRAM, DVE
             reciprocal, partition-broadcast, in-place normalize.
  Epilogue:  out = ctxT.T @ Wp (+bp)          (fp32r)

kernel() accepts the FULL inputs and returns the FULL output.
"""

import numpy as np

B, S, HID, NH = 8, 1024, 1024, 16
HD = HID // NH          # 64
P = 128                 # partitions
ST = S // P             # 8 s-tiles
HT = HID // P           # 8 hid-tiles
QC = S // 512           # 2 free-dim chunks of 512
N_CORES = 8

_BUILT = {}


def _build(with_bias):
    from concourse import bass, bacc, mybir, tile
    from concourse.masks import make_identity

    f32 = mybir.dt.float32
    f32r = mybir.dt.float32r
    bf16 = mybir.dt.bfloat16
    Alu = mybir.AluOpType
    Act = mybir.ActivationFunctionType

    nc = bacc.Bacc("TRN2", target_bir_lowering=False, debug=False,
                   num_devices=N_CORES)

    x_d = nc.declare_dram_parameter("x", [S, HID], f32, isOutput=False)
    mask_d = nc.declare_dram_parameter("mask", [S, S], f32, isOutput=False)
    wq_d = nc.declare_dram_parameter("wq", [HID, HID], f32, isOutput=False)
    wk_d = nc.declare_dram_parameter("wk", [HID, HID], f32, isOutput=False)
    wv_d = nc.declare_dram_parameter("wv", [HID, HID], f32, isOutput=False)
    wp_d = nc.declare_dram_parameter("wp", [HID, HID], f32, isOutput=False)
    if with_bias:
        bq_d = nc.declare_dram_parameter("bq", [1, HID], f32, isOutput=False)
        bk_d = nc.declare_dram_parameter("bk", [1, HID], f32, isOutput=False)
        bv_d = nc.declare_dram_parameter("bv", [1, HID], f32, isOutput=False)
        bp_d = nc.declare_dram_parameter("bp", [1, HID], f32, isOutput=False)
    out_d = nc.declare_dram_parameter("out", [S, HID], f32, isOutput=True)

    def r(ap):
        return ap.bitcast(f32r)

    with tile.TileContext(nc) as tc:
        # ---- pools (stack-ordered per side) ----
        const = tc.alloc_tile_pool(name="const", bufs=1, side="left")
        qtp = tc.alloc_tile_pool(name="qtp", bufs=1, side="left")
        ktp = tc.alloc_tile_pool(name="ktp", bufs=1, side="left")
        vpp = tc.alloc_tile_pool(name="vpp", bufs=1, side="left")
        ep = tc.alloc_tile_pool(name="ep", bufs=1, side="left")
        xTp = tc.alloc_tile_pool(name="xTp", bufs=1, side="left")
        wqbp = tc.alloc_tile_pool(name="wqbp", bufs=8, side="left")
        wkbp = tc.alloc_tile_pool(name="wkbp", bufs=8, side="left")
        # prologue-only pools (released before attention)
        wvbp = tc.alloc_tile_pool(name="wvbp", bufs=8, side="left")
        mtp = tc.alloc_tile_pool(name="mtp", bufs=8, side="left")
        minp = tc.alloc_tile_pool(name="minp", bufs=2, side="left")
        xload = tc.alloc_tile_pool(name="xload", bufs=2, side="left")
        wstg = tc.alloc_tile_pool(name="wstg", bufs=4, side="right")
        tpsum = tc.alloc_tile_pool(name="tpsum", bufs=2, space="PSUM")
        qkvpsum = tc.alloc_tile_pool(name="qkvpsum", bufs=4, space="PSUM")

        ident = const.tile([P, P], f32)
        make_identity(nc, ident)
        if with_bias:
            ones_row = const.tile([1, 512], f32r)
            nc.vector.memset(ones_row[:], 1.0)
            ones_bf = const.tile([1, 512], bf16)
            nc.vector.memset(ones_bf[:], 1.0)
            bias_sb = const.tile([4, HID], f32r)
            nc.sync.dma_start(bias_sb[0:1, :], bq_d[:].bitcast(f32r))
            nc.sync.dma_start(bias_sb[1:2, :], bk_d[:].bitcast(f32r))
            nc.sync.dma_start(bias_sb[2:3, :], bv_d[:].bitcast(f32r))
            nc.sync.dma_start(bias_sb[3:4, :], bp_d[:].bitcast(f32r))
            bias_bf = const.tile([4, HID], bf16)
            nc.scalar.copy(bias_bf[:], bias_sb[:].bitcast(f32))

        QT = qtp.tile([P, HT, S], bf16)              # QT[p, j, s] = Q[s, j*128+p]
        KT = ktp.tile([P, HT, S], bf16)
        Vp = vpp.tile([P, ST, NH, HD + 1], bf16)     # V'[p, si, h, c]
        E = ep.tile([P, ST, S], bf16)                # E[p, ki, q] = exp-mask
        xT = xTp.tile([P, HT, S], bf16)              # xT[p, j, s] = x[s, j*128+p]

        nc.vector.memset(Vp[:, :, :, HD:HD + 1], 1.0)

        # ---- prologue: load x, transpose to xT (bf16) ----
        for si in range(ST):
            xs = xload.tile([P, HID], f32, name="xs")
            # alternate the two HWDGE engines (SP / Activation) so x tiles
            # arrive two-at-a-time instead of pacing the transposes
            dma_eng = nc.sync if si % 2 == 0 else nc.scalar
            dma_eng.dma_start(xs[:], x_d[si * P:(si + 1) * P, :])
            for g in range(2):  # groups of 4 hid-tiles
                tp = tpsum.tile([P, 512], f32, name="tp")
                for u in range(4):
                    hj = g * 4 + u
                    nc.tensor.transpose(tp[:, u * P:(u + 1) * P],
                                        xs[:, hj * P:(hj + 1) * P], ident[:])
                nc.scalar.copy(
                    xT[:, g * 4:(g + 1) * 4, si * P:(si + 1) * P],
                    tp[:].rearrange("p (a b) -> p a b", a=4))

        # ---- weights: load f32, convert to bf16 on-chip ----
        def load_w_bf(dram, pool, dma_eng):
            tiles = []
            for kj in range(HT):
                ws = wstg.tile([P, HID], f32, name="ws")
                dma_eng.dma_start(ws[:], dram[kj * P:(kj + 1) * P, :])
                wb = pool.tile([P, HID], bf16, name="wb")
                nc.vector.tensor_copy(wb[:], ws[:])
                tiles.append(wb)
            return tiles

        # ---- mask -> E: DMA + rowmin/scale (DVE), transpose (PE), exp (ACT)
        # (mask DMAs and DVE prep run BEFORE the weight converts so the PE's
        # mask transposes aren't gated behind 24 convert ops in the DVE queue)
        mts = []
        for qi in range(ST):
            mt = mtp.tile([P, S], f32, name="mt")
            nc.scalar.dma_start(mt[:], mask_d[qi * P:(qi + 1) * P, :])
            mn = minp.tile([P, 1], f32, name="mn")
            nc.vector.tensor_reduce(mn[:], mt[:], axis=mybir.AxisListType.X,
                                    op=Alu.min)
            # mt = (mask - rowmin) * (-1e9)
            nc.vector.tensor_scalar(mt[:], mt[:], mn[:], -1.0e9,
                                    op0=Alu.subtract, op1=Alu.mult)
            mts.append(mt)

        wqb = load_w_bf(wq_d, wqbp, nc.sync)
        wkb = load_w_bf(wk_d, wkbp, nc.scalar)
        wvb = load_w_bf(wv_d, wvbp, nc.sync)

        for qi in range(ST):
            mt = mts[qi]
            for g in range(2):
                tp = tpsum.tile([P, 512], f32, name="tp")
                for u in range(4):
                    ki = g * 4 + u
                    nc.tensor.transpose(tp[:, u * P:(u + 1) * P],
                                        mt[:, ki * P:(ki + 1) * P], ident[:])
                nc.scalar.activation(
                    E[:, g * 4:(g + 1) * 4, qi * P:(qi + 1) * P],
                    tp[:].rearrange("p (a b) -> p a b", a=4),
                    Act.Exp, bias=0.0, scale=1.0)

        # ---- prologue projections: QT/KT for head pair 0, V for all ----
        def qk_group(dst, wtiles, brow, dj, sc, pool):
            ps = pool.tile([P, 512], f32, name="ps")
            for kj in range(HT):
                nc.tensor.matmul(
                    ps[:],
                    wtiles[kj][:, dj * P:(dj + 1) * P],
                    xT[:, kj, sc * 512:(sc + 1) * 512],
                    start=(kj == 0), stop=(kj == HT - 1 and not with_bias))
            if with_bias:
                nc.tensor.matmul(
                    ps[:],
                    bias_bf[brow:brow + 1, dj * P:(dj + 1) * P],
                    ones_bf[:],
                    start=False, stop=True)
            nc.vector.tensor_copy(dst[:, dj, sc * 512:(sc + 1) * 512], ps[:])

        PAIRS = [1, 2, 3, 4, 5, 6, 7, 0]  # pair processing order; pair 0
        # last so the epilogue's hj=0 contraction step is the only one
        # gated on the final head's normalize
        for sc in range(QC):
            qk_group(QT, wqb, 0, PAIRS[0], sc, qkvpsum)
            qk_group(KT, wkb, 1, PAIRS[0], sc, qkvpsum)

        for si in range(ST):
            for dc in range(QC):
                ps = qkvpsum.tile([P, 512], f32, name="ps")
                for kj in range(HT):
                    nc.tensor.matmul(
                        ps[:],
                        xT[:, kj, si * P:(si + 1) * P],
                        wvb[kj][:, dc * 512:(dc + 1) * 512],
                        start=(kj == 0), stop=(kj == HT - 1 and not with_bias))
                if with_bias:
                    nc.tensor.matmul(
                        ps[:],
                        ones_bf[:, 0:P],
                        bias_bf[2:3, dc * 512:(dc + 1) * 512],
                        start=False, stop=True)
                nc.vector.tensor_copy(
                    Vp[:, si, dc * 8:(dc + 1) * 8, 0:HD],
                    ps[:].rearrange("p (h c) -> p h c", h=8))

        qkvpsum.release()
        tpsum.release()
        wstg.release()
        xload.release()
        minp.release()
        mtp.release()
        wvbp.release()

        # ---- attention + interleaved QT/KT filler ----
        wpool = tc.alloc_tile_pool(name="wpool", bufs=8, side="right")
        ctxp = tc.alloc_tile_pool(name="ctxp", bufs=1, side="right")
        scpsum = tc.alloc_tile_pool(name="scpsum", bufs=3, space="PSUM")
        cxpsum = tc.alloc_tile_pool(name="cxpsum", bufs=2, space="PSUM")
        fillps = tc.alloc_tile_pool(name="fillps", bufs=1, space="PSUM")
        apool = tc.alloc_tile_pool(name="apool", bufs=4, side="right")
        stgp = tc.alloc_tile_pool(name="stgp", bufs=1, side="right")
        dnp = tc.alloc_tile_pool(name="dnp", bufs=2, side="right")
        rhp = tc.alloc_tile_pool(name="rhp", bufs=1, side="right")
        rbp = tc.alloc_tile_pool(name="rbp", bufs=2, side="right")
        dpool = tc.alloc_tile_pool(name="dpool", bufs=1, space="DRAM")

        ctxT = ctxp.tile([P, HT, S], f32r)           # ctxT[p, j, q]
        dscr = dpool.tile([1, NH * S], f32, name="dscr")
        dscr2 = dpool.tile([1, NH * S], f32, name="dscr2")

        # filler: QT/KT projections for head pair j+1, one closure per MM
        def mk_fill(dst, wtiles, brow, dj, sc, kj, holder):
            def go():
                if kj == 0:
                    holder["ps"] = fillps.tile([P, 512], f32, name="fps")
                ps = holder["ps"]
                nc.tensor.matmul(
                    ps[:],
                    wtiles[kj][:, dj * P:(dj + 1) * P],
                    xT[:, kj, sc * 512:(sc + 1) * 512],
                    start=(kj == 0), stop=(kj == HT - 1 and not with_bias))
                if kj == HT - 1:
                    if with_bias:
                        nc.tensor.matmul(
                            ps[:],
                            bias_bf[brow:brow + 1, dj * P:(dj + 1) * P],
                            ones_bf[:],
                            start=False, stop=True)
                    nc.scalar.copy(dst[:, dj, sc * 512:(sc + 1) * 512], ps[:])
            return go

        flat = []
        bstart = [0] * 8  # batch b -> start index in flat
        for b in range(7):
            bstart[b] = len(flat)
            for dst, wt, brow in ((QT, wqb, 0), (KT, wkb, 1)):
                for sc in range(QC):
                    holder = {}
                    for kj in range(HT):
                        flat.append(mk_fill(dst, wt, brow, PAIRS[b + 1], sc,
                                            kj, holder))
        bstart[7] = len(flat)
        fi = [0]

        def fill(n):
            while n > 0 and fi[0] < len(flat):
                flat[fi[0]]()
                fi[0] += 1
                n -= 1

        def flush(j):
            end = bstart[j + 1] if j + 1 < len(bstart) else len(flat)
            while fi[0] < end:
                flat[fi[0]]()
                fi[0] += 1

        LAG = 2  # ktiles the CX matmuls trail the score matmuls by

        def head_tail(h, CX):
            # evict unnormalized ctx right away (frees the CX PSUM bank),
            # then run the reciprocal chain off the critical path and
            # normalize in place in SBUF
            j, po = h // 2, (h % 2) * 64
            stg = stgp.tile([1, S], f32, name="stg")
            nc.vector.tensor_copy(stg[:], CX[HD:HD + 1, :])
            nc.vector.tensor_copy(ctxT[po:po + 64, j, :], CX[0:HD, :])
            nc.sync.dma_start(dscr[0:1, h * S:(h + 1) * S], stg[:])
            dn = dnp.tile([8, P], f32, name="dn")
            nc.sync.dma_start(
                dn[:],
                dscr[0:1, h * S:(h + 1) * S].rearrange(
                    "p (a b) -> (p a) b", a=8))
            nc.vector.reciprocal(dn[:], dn[:])
            nc.sync.dma_start(
                dscr2[0:1, h * S:(h + 1) * S].rearrange(
                    "p (a b) -> (p a) b", a=8),
                dn[:])
            rh = rhp.tile([1, S], f32, name="rh")
            nc.sync.dma_start(rh[:], dscr2[0:1, h * S:(h + 1) * S])
            RB = rbp.tile([P, S], f32, name="RB")
            nc.gpsimd.partition_broadcast(RB[:], rh[:])
            nc.vector.tensor_tensor(
                ctxT[po:po + 64, j, :],
                ctxT[po:po + 64, j, :].bitcast(f32),
                RB[po:po + 64, :],
                op=Alu.mult)

        def head_of(t):
            return 2 * PAIRS[t // 16] + (t // ST) % 2

        def issue_cx(t):
            h, ki = head_of(t), t % ST
            A, CX = pend[t]
            for qc in range(QC):
                nc.tensor.matmul(
                    CX[0:HD + 1, qc * 512:(qc + 1) * 512],
                    Vp[:, ki, h, :],
                    A[:, qc * 512:(qc + 1) * 512],
                    start=(ki == 0), stop=(ki == ST - 1))
            if ki == ST - 1:
                head_tail(h, CX)

        pend = {}
        partials = {}
        CX = None
        for t in range(NH * ST):  # flat (head, ktile) stream
            h, ki = head_of(t), t % ST
            j, po = h // 2, (h % 2) * 64
            if t % 16 == 0 and t > 0:
                flush(t // 16 - 1)  # QT/KT for this pair must be complete
            if ki == 0:
                CX = cxpsum.tile([P, S], f32, name="CX")
            if t == 80:  # prefetch Wp once filler pressure eases
                wptiles = [wpool.tile([P, HID], f32r, name="wt")
                           for _ in range(HT)]
                for kj in range(HT):
                    nc.sync.dma_start(
                        wptiles[kj][:],
                        wp_d[kj * P:(kj + 1) * P, :].bitcast(f32r))
            A = apool.tile([P, S], bf16, name="A")
            sc_chunks = []
            for qc in range(QC):
                Sc = scpsum.tile([P, 512], f32, name="Sc")
                nc.tensor.matmul(
                    Sc[:],
                    KT[po:po + 64, j, ki * P:(ki + 1) * P],
                    QT[po:po + 64, j, qc * 512:(qc + 1) * 512],
                    start=True, stop=True)
                sc_chunks.append(Sc)
            if t - LAG >= 0:
                issue_cx(t - LAG)
            fill(2)
            for qc in range(QC):
                nc.scalar.activation(
                    A[:, qc * 512:(qc + 1) * 512], sc_chunks[qc][:],
                    Act.Exp, bias=0.0, scale=float(1.0 / 32.0))
            nc.vector.tensor_tensor(A[:], A[:], E[:, ki, :], op=Alu.mult)
            pend[t] = (A, CX)
        for t in range(NH * ST - LAG, NH * ST):
            issue_cx(t)

        dpool.release()
        rbp.release()
        rhp.release()
        dnp.release()
        stgp.release()
        apool.release()
        fillps.release()
        cxpsum.release()
        scpsum.release()

        # ---- epilogue: output projection ----
        prpsum = tc.alloc_tile_pool(name="prpsum", bufs=4, space="PSUM")
        opool = tc.alloc_tile_pool(name="opool", bufs=3, side="right")
        for qi in range(ST):
            for dc in range(QC):
                ps = prpsum.tile([P, 512], f32, name="pps")
                hjs = PAIRS[6:] if (qi, dc) in partials else PAIRS
                for n, hj in enumerate(hjs):
                    nc.tensor.matmul(
                        ps[:],
                        r(ctxT[:, hj, qi * P:(qi + 1) * P]),
                        r(wptiles[hj][:, dc * 512:(dc + 1) * 512]),
                        start=(n == 0),
                        stop=(n == len(hjs) - 1 and not with_bias))
                if with_bias:
                    nc.tensor.matmul(
                        ps[:],
                        r(ones_row[:, 0:P]),
                        r(bias_sb[3:4, dc * 512:(dc + 1) * 512]),
                        start=False, stop=True)
                osb = opool.tile([P, 512], f32, name="osb")
                if (qi, dc) in partials:
                    nc.vector.tensor_tensor(
                        osb[:], ps[:], partials[(qi, dc)][:], op=Alu.add)
                else:
                    nc.vector.tensor_copy(osb[:], ps[:])
                nc.sync.dma_start(
                    out_d[qi * P:(qi + 1) * P, dc * 512:(dc + 1) * 512], osb[:])
        opool.release()
        prpsum.release()
        ctxp.release()
        wpool.release()
        wkbp.release()
        wqbp.release()
        xTp.release()
        ep.release()
        vpp.release()
        ktp.release()
        qtp.release()
        const.release()

    nc.compile()
    return nc


def _get(with_bias):
    if with_bias not in _BUILT:
        _BUILT[with_bias] = _build(with_bias)
    return _BUILT[with_bias]


def _make_in_maps(inputs, with_bias):
    f = lambda a: np.ascontiguousarray(np.asarray(a), dtype=np.float32)
    x = f(inputs["x"])
    mask = f(inputs["attention_mask"])
    shared = {
        "wq": f(inputs["Wq"]), "wk": f(inputs["Wk"]),
        "wv": f(inputs["Wv"]), "wp": f(inputs["Wp"]),
    }
    if with_bias:
        shared["bq"] = f(inputs["bq"]).reshape(1, HID)
        shared["bk"] = f(inputs["bk"]).reshape(1, HID)
        shared["bv"] = f(inputs["bv"]).reshape(1, HID)
        shared["bp"] = f(inputs["bp"]).reshape(1, HID)
    return [
        dict(shared, x=x[b], mask=np.ascontiguousarray(mask[b, 0]))
        for b in range(N_CORES)
    ]


def run(trace=False, **inputs):
    from concourse.bass_utils import run_bass_kernel_spmd
    with_bias = any(
        float(np.abs(np.asarray(inputs[k])).max()) != 0.0
        for k in ("bq", "bk", "bv", "bp"))
    nc = _get(with_bias)
    in_maps = _make_in_maps(inputs, with_bias)
    res = run_bass_kernel_spmd(nc, in_maps, list(range(N_CORES)), trace=trace)
    out = np.stack([res.results[i]["out"] for i in range(N_CORES)])
    return out.astype(np.float32, copy=False), res


def kernel(**inputs):
    out, _ = run(trace=False, **inputs)
    return out


# revision 58
# speedup vs baseline: 1.0241x; 1.0241x over previous
"""Trainium2 Bass kernel for nn_MultiHeadAttention (B=8, S=1024, HID=1024, NH=16).

Strategy: data-parallel over batch — core b computes the full MHA for batch
element b (B == n_cores == 8, no collectives).

The kernel is organized to keep the PE (tensor engine) queue dense end-to-end
so the HAM clock gate stays at 2.4 GHz (scattered micro-idles re-throttle it
to 1.2 GHz, doubling every matmul):

  Prologue:  xT = x.T (PE transpose, bf16)
             E  = bf16 exp(-1e9*(mask - rowmin(mask))).T   (shared by heads)
             wq/wk/wv converted to bf16 on-chip
             QT/KT for head pair 0 (d-major, bf16);  V for ALL heads (bf16,
             s-major, +ones column per head for the softmax denominator)
  Attention: flat stream over (head, ktile); per ktile:
               S[k,q] = KT_h.T @ QT_h         (bf16, PSUM, 2x 512 chunks)
               A      = exp(S * 1/32)         (ACT, bf16)
               A     *= E[ki]                 (DVE bf16 2x mode)
               CX    += V'_h[ki].T @ A        (bf16; row 64 = denom)
             CX matmuls lag the score matmuls by 2 ktiles; QT/KT projection
             matmuls for head pair j+1 are interleaved as PE filler during
             pair j so the PE never idles.
             Per-head tail (DVE/DMA/gpsimd, off critical path): evict
             unnormalized ctx, denom -> [8,128] reshape via DRAM, DVE
             reciprocal, partition-broadcast, in-place normalize.
  Epilogue:  out = ctxT.T @ Wp (+bp)          (fp32r)

kernel() accepts the FULL inputs and returns the FULL output.
"""

import numpy as np

B, S, HID, NH = 8, 1024, 1024, 16
HD = HID // NH          # 64
P = 128                 # partitions
ST = S // P             # 8 s-tiles
HT = HID // P           # 8 hid-tiles
QC = S // 512           # 2 free-dim chunks of 512
N_CORES = 8

_BUILT = {}


def _build(with_bias):
    from concourse import bass, bacc, mybir, tile
    from concourse.masks import make_identity

    f32 = mybir.dt.float32
    f32r = mybir.dt.float32r
    bf16 = mybir.dt.bfloat16
    Alu = mybir.AluOpType
    Act = mybir.ActivationFunctionType

    nc = bacc.Bacc("TRN2", target_bir_lowering=False, debug=False,
                   num_devices=N_CORES)

    x_d = nc.declare_dram_parameter("x", [S, HID], f32, isOutput=False)
    mask_d = nc.declare_dram_parameter("mask", [S, S], f32, isOutput=False)
    wq_d = nc.declare_dram_parameter("wq", [HID, HID], f32, isOutput=False)
    wk_d = nc.declare_dram_parameter("wk", [HID, HID], f32, isOutput=False)
    wv_d = nc.declare_dram_parameter("wv", [HID, HID], f32, isOutput=False)
    wp_d = nc.declare_dram_parameter("wp", [HID, HID], f32, isOutput=False)
    if with_bias:
        bq_d = nc.declare_dram_parameter("bq", [1, HID], f32, isOutput=False)
        bk_d = nc.declare_dram_parameter("bk", [1, HID], f32, isOutput=False)
        bv_d = nc.declare_dram_parameter("bv", [1, HID], f32, isOutput=False)
        bp_d = nc.declare_dram_parameter("bp", [1, HID], f32, isOutput=False)
    out_d = nc.declare_dram_parameter("out", [S, HID], f32, isOutput=True)

    def r(ap):
        return ap.bitcast(f32r)

    with tile.TileContext(nc) as tc:
        # ---- pools (stack-ordered per side) ----
        const = tc.alloc_tile_pool(name="const", bufs=1, side="left")
        qtp = tc.alloc_tile_pool(name="qtp", bufs=1, side="left")
        ktp = tc.alloc_tile_pool(name="ktp", bufs=1, side="left")
        vpp = tc.alloc_tile_pool(name="vpp", bufs=1, side="left")
        ep = tc.alloc_tile_pool(name="ep", bufs=1, side="left")
        xTp = tc.alloc_tile_pool(name="xTp", bufs=1, side="left")
        wqbp = tc.alloc_tile_pool(name="wqbp", bufs=8, side="left")
        wkbp = tc.alloc_tile_pool(name="wkbp", bufs=8, side="left")
        # prologue-only pools (released before attention)
        wvbp = tc.alloc_tile_pool(name="wvbp", bufs=8, side="left")
        mtp = tc.alloc_tile_pool(name="mtp", bufs=8, side="left")
        minp = tc.alloc_tile_pool(name="minp", bufs=2, side="left")
        xload = tc.alloc_tile_pool(name="xload", bufs=2, side="left")
        wstg = tc.alloc_tile_pool(name="wstg", bufs=4, side="right")
        tpsum = tc.alloc_tile_pool(name="tpsum", bufs=2, space="PSUM")
        qkvpsum = tc.alloc_tile_pool(name="qkvpsum", bufs=4, space="PSUM")

        ident = const.tile([P, P], f32)
        make_identity(nc, ident)
        if with_bias:
            ones_row = const.tile([1, 512], f32r)
            nc.vector.memset(ones_row[:], 1.0)
            ones_bf = const.tile([1, 512], bf16)
            nc.vector.memset(ones_bf[:], 1.0)
            bias_sb = const.tile([4, HID], f32r)
            nc.sync.dma_start(bias_sb[0:1, :], bq_d[:].bitcast(f32r))
            nc.sync.dma_start(bias_sb[1:2, :], bk_d[:].bitcast(f32r))
            nc.sync.dma_start(bias_sb[2:3, :], bv_d[:].bitcast(f32r))
            nc.sync.dma_start(bias_sb[3:4, :], bp_d[:].bitcast(f32r))
            bias_bf = const.tile([4, HID], bf16)
            nc.scalar.copy(bias_bf[:], bias_sb[:].bitcast(f32))

        QT = qtp.tile([P, HT, S], bf16)              # QT[p, j, s] = Q[s, j*128+p]
        KT = ktp.tile([P, HT, S], bf16)
        Vp = vpp.tile([P, ST, NH, HD + 1], bf16)     # V'[p, si, h, c]
        E = ep.tile([P, ST, S], bf16)                # E[p, ki, q] = exp-mask
        xT = xTp.tile([P, HT, S], bf16)              # xT[p, j, s] = x[s, j*128+p]

        nc.vector.memset(Vp[:, :, :, HD:HD + 1], 1.0)

        # ---- prologue: load x, transpose to xT (bf16) ----
        for si in range(ST):
            xs = xload.tile([P, HID], f32, name="xs")
            # alternate the two HWDGE engines (SP / Activation) so x tiles
            # arrive two-at-a-time instead of pacing the transposes
            dma_eng = nc.sync if si % 2 == 0 else nc.scalar
            dma_eng.dma_start(xs[:], x_d[si * P:(si + 1) * P, :])
            for g in range(2):  # groups of 4 hid-tiles
                tp = tpsum.tile([P, 512], f32, name="tp")
                for u in range(4):
                    hj = g * 4 + u
                    nc.tensor.transpose(tp[:, u * P:(u + 1) * P],
                                        xs[:, hj * P:(hj + 1) * P], ident[:])
                nc.scalar.copy(
                    xT[:, g * 4:(g + 1) * 4, si * P:(si + 1) * P],
                    tp[:].rearrange("p (a b) -> p a b", a=4))

        # ---- weights: load f32, convert to bf16 on-chip ----
        def load_w_bf(dram, pool, dma_eng):
            tiles = []
            for kj in range(HT):
                ws = wstg.tile([P, HID], f32, name="ws")
                dma_eng.dma_start(ws[:], dram[kj * P:(kj + 1) * P, :])
                wb = pool.tile([P, HID], bf16, name="wb")
                nc.vector.tensor_copy(wb[:], ws[:])
                tiles.append(wb)
            return tiles

        # ---- mask -> E: DMA + rowmin/scale (DVE), transpose (PE), exp (ACT)
        # (mask DMAs and DVE prep run BEFORE the weight converts so the PE's
        # mask transposes aren't gated behind 24 convert ops in the DVE queue)
        mts = []
        for qi in range(ST):
            mt = mtp.tile([P, S], f32, name="mt")
            nc.scalar.dma_start(mt[:], mask_d[qi * P:(qi + 1) * P, :])
            mn = minp.tile([P, 1], f32, name="mn")
            nc.vector.tensor_reduce(mn[:], mt[:], axis=mybir.AxisListType.X,
                                    op=Alu.min)
            # mt = (mask - rowmin) * (-1e9)
            nc.vector.tensor_scalar(mt[:], mt[:], mn[:], -1.0e9,
                                    op0=Alu.subtract, op1=Alu.mult)
            mts.append(mt)

        wqb = load_w_bf(wq_d, wqbp, nc.sync)
        wkb = load_w_bf(wk_d, wkbp, nc.scalar)
        wvb = load_w_bf(wv_d, wvbp, nc.sync)

        for qi in range(ST):
            mt = mts[qi]
            for g in range(2):
                tp = tpsum.tile([P, 512], f32, name="tp")
                for u in range(4):
                    ki = g * 4 + u
                    nc.tensor.transpose(tp[:, u * P:(u + 1) * P],
                                        mt[:, ki * P:(ki + 1) * P], ident[:])
                nc.scalar.activation(
                    E[:, g * 4:(g + 1) * 4, qi * P:(qi + 1) * P],
                    tp[:].rearrange("p (a b) -> p a b", a=4),
                    Act.Exp, bias=0.0, scale=1.0)

        # ---- prologue projections: QT/KT for head pair 0, V for all ----
        def qk_group(dst, wtiles, brow, dj, sc, pool):
            ps = pool.tile([P, 512], f32, name="ps")
            for kj in range(HT):
                nc.tensor.matmul(
                    ps[:],
                    wtiles[kj][:, dj * P:(dj + 1) * P],
                    xT[:, kj, sc * 512:(sc + 1) * 512],
                    start=(kj == 0), stop=(kj == HT - 1 and not with_bias))
            if with_bias:
                nc.tensor.matmul(
                    ps[:],
                    bias_bf[brow:brow + 1, dj * P:(dj + 1) * P],
                    ones_bf[:],
                    start=False, stop=True)
            nc.vector.tensor_copy(dst[:, dj, sc * 512:(sc + 1) * 512], ps[:])

        PAIRS = [1, 2, 3, 4, 5, 6, 7, 0]  # pair processing order; pair 0
        # last so the epilogue's hj=0 contraction step is the only one
        # gated on the final head's normalize
        for sc in range(QC):
            qk_group(QT, wqb, 0, PAIRS[0], sc, qkvpsum)
            qk_group(KT, wkb, 1, PAIRS[0], sc, qkvpsum)

        for si in range(ST):
            for dc in range(QC):
                ps = qkvpsum.tile([P, 512], f32, name="ps")
                for kj in range(HT):
                    nc.tensor.matmul(
                        ps[:],
                        xT[:, kj, si * P:(si + 1) * P],
                        wvb[kj][:, dc * 512:(dc + 1) * 512],
                        start=(kj == 0), stop=(kj == HT - 1 and not with_bias))
                if with_bias:
                    nc.tensor.matmul(
                        ps[:],
                        ones_bf[:, 0:P],
                        bias_bf[2:3, dc * 512:(dc + 1) * 512],
                        start=False, stop=True)
                nc.vector.tensor_copy(
                    Vp[:, si, dc * 8:(dc + 1) * 8, 0:HD],
                    ps[:].rearrange("p (h c) -> p h c", h=8))

        qkvpsum.release()
        tpsum.release()
        wstg.release()
        xload.release()
        minp.release()
        mtp.release()
        wvbp.release()

        # ---- attention + interleaved QT/KT filler ----
        wpool = tc.alloc_tile_pool(name="wpool", bufs=8, side="right")
        ctxp = tc.alloc_tile_pool(name="ctxp", bufs=1, side="right")
        scpsum = tc.alloc_tile_pool(name="scpsum", bufs=3, space="PSUM")
        cxpsum = tc.alloc_tile_pool(name="cxpsum", bufs=2, space="PSUM")
        fillps = tc.alloc_tile_pool(name="fillps", bufs=1, space="PSUM")
        apool = tc.alloc_tile_pool(name="apool", bufs=4, side="right")
        stgp = tc.alloc_tile_pool(name="stgp", bufs=1, side="right")
        dnp = tc.alloc_tile_pool(name="dnp", bufs=2, side="right")
        rhp = tc.alloc_tile_pool(name="rhp", bufs=1, side="right")
        rbp = tc.alloc_tile_pool(name="rbp", bufs=2, side="right")
        dpool = tc.alloc_tile_pool(name="dpool", bufs=1, space="DRAM")

        ctxT = ctxp.tile([P, HT, S], f32r)           # ctxT[p, j, q]
        dscr = dpool.tile([1, NH * S], f32, name="dscr")
        dscr2 = dpool.tile([1, NH * S], f32, name="dscr2")

        # dummy broadcast: pre-load the gpsimd partition_broadcast library
        # now, not in the first head's normalize chain (LIBRARY_RELOAD there
        # stalled the DVE FIFO long enough to re-throttle the HAM clock)
        rh0 = rhp.tile([1, S], f32, name="rh")
        nc.vector.memset(rh0[:], 1.0)
        rb0 = rbp.tile([P, S], f32, name="RB")
        nc.gpsimd.partition_broadcast(rb0[:], rh0[:])

        # filler: QT/KT projections for head pair j+1, one closure per MM
        def mk_fill(dst, wtiles, brow, dj, sc, kj, holder):
            def go():
                if kj == 0:
                    holder["ps"] = fillps.tile([P, 512], f32, name="fps")
                ps = holder["ps"]
                nc.tensor.matmul(
                    ps[:],
                    wtiles[kj][:, dj * P:(dj + 1) * P],
                    xT[:, kj, sc * 512:(sc + 1) * 512],
                    start=(kj == 0), stop=(kj == HT - 1 and not with_bias))
                if kj == HT - 1:
                    if with_bias:
                        nc.tensor.matmul(
                            ps[:],
                            bias_bf[brow:brow + 1, dj * P:(dj + 1) * P],
                            ones_bf[:],
                            start=False, stop=True)
                    nc.scalar.copy(dst[:, dj, sc * 512:(sc + 1) * 512], ps[:])
            return go

        flat = []
        bstart = [0] * 8  # batch b -> start index in flat
        for b in range(7):
            bstart[b] = len(flat)
            for dst, wt, brow in ((QT, wqb, 0), (KT, wkb, 1)):
                for sc in range(QC):
                    holder = {}
                    for kj in range(HT):
                        flat.append(mk_fill(dst, wt, brow, PAIRS[b + 1], sc,
                                            kj, holder))
        bstart[7] = len(flat)
        fi = [0]

        def fill(n):
            while n > 0 and fi[0] < len(flat):
                flat[fi[0]]()
                fi[0] += 1
                n -= 1

        def flush(j):
            end = bstart[j + 1] if j + 1 < len(bstart) else len(flat)
            while fi[0] < end:
                flat[fi[0]]()
                fi[0] += 1

        LAG = 2  # ktiles the CX matmuls trail the score matmuls by

        # Per-head tail steps run as deferred closures with due-times. The
        # normalize (c4) waits on the whole reciprocal/broadcast chain; if it
        # enters the DVE FIFO too early it blocks every queued A-multiply
        # behind it, starving the CX matmuls and re-throttling the HAM clock.
        tail_q = []  # (due_t, closure), appended in due order
        tail_state = {}

        def drain_tails(t):
            while tail_q and tail_q[0][0] <= t:
                tail_q.pop(0)[1]()

        def head_tail(h, CX, t_now):
            j, po = h // 2, (h % 2) * 64

            def c1():  # denom row -> SBUF -> DRAM -> [8,128]
                stg = stgp.tile([1, S], f32, name="stg")
                nc.vector.tensor_copy(stg[:], CX[HD:HD + 1, :])
                nc.sync.dma_start(dscr[0:1, h * S:(h + 1) * S], stg[:])
                dn = dnp.tile([8, P], f32, name="dn")
                nc.sync.dma_start(
                    dn[:],
                    dscr[0:1, h * S:(h + 1) * S].rearrange(
                        "p (a b) -> (p a) b", a=8))
                tail_state[h] = dn

            def c2():  # evict unnormalized ctx (frees the CX PSUM bank)
                nc.vector.tensor_copy(ctxT[po:po + 64, j, :], CX[0:HD, :])

            def c3():  # reciprocal + reshape back + broadcast
                dn = tail_state[h]
                nc.vector.reciprocal(dn[:], dn[:])
                nc.sync.dma_start(
                    dscr2[0:1, h * S:(h + 1) * S].rearrange(
                        "p (a b) -> (p a) b", a=8),
                    dn[:])
                rh = rhp.tile([1, S], f32, name="rh")
                nc.sync.dma_start(rh[:], dscr2[0:1, h * S:(h + 1) * S])
                RB = rbp.tile([P, S], f32, name="RB")
                nc.gpsimd.partition_broadcast(RB[:], rh[:])
                tail_state[h] = RB

            def c4():  # normalize in place (chain has finished by now)
                RB = tail_state.pop(h)
                nc.vector.tensor_tensor(
                    ctxT[po:po + 64, j, :],
                    ctxT[po:po + 64, j, :].bitcast(f32),
                    RB[po:po + 64, :],
                    op=Alu.mult)

            tail_q.extend([(t_now + 1, c1), (t_now + 2, c2),
                           (t_now + 3, c3), (t_now + 7, c4)])

        def head_of(t):
            return 2 * PAIRS[t // 16] + (t // ST) % 2

        def issue_cx(t, t_now):
            h, ki = head_of(t), t % ST
            A, CX = pend[t]
            for qc in range(QC):
                nc.tensor.matmul(
                    CX[0:HD + 1, qc * 512:(qc + 1) * 512],
                    Vp[:, ki, h, :],
                    A[:, qc * 512:(qc + 1) * 512],
                    start=(ki == 0), stop=(ki == ST - 1))
            if ki == ST - 1:
                head_tail(h, CX, t_now)

        pend = {}
        partials = {}
        CX = None
        for t in range(NH * ST):  # flat (head, ktile) stream
            h, ki = head_of(t), t % ST
            j, po = h // 2, (h % 2) * 64
            if t % 16 == 0 and t > 0:
                flush(t // 16 - 1)  # QT/KT for this pair must be complete
            if ki == 0:
                CX = cxpsum.tile([P, S], f32, name="CX")
            if t == 80:  # prefetch Wp once filler pressure eases
                wptiles = [wpool.tile([P, HID], f32r, name="wt")
                           for _ in range(HT)]
                for kj in range(HT):
                    nc.sync.dma_start(
                        wptiles[kj][:],
                        wp_d[kj * P:(kj + 1) * P, :].bitcast(f32r))
            A = apool.tile([P, S], bf16, name="A")
            sc_chunks = []
            for qc in range(QC):
                Sc = scpsum.tile([P, 512], f32, name="Sc")
                nc.tensor.matmul(
                    Sc[:],
                    KT[po:po + 64, j, ki * P:(ki + 1) * P],
                    QT[po:po + 64, j, qc * 512:(qc + 1) * 512],
                    start=True, stop=True)
                sc_chunks.append(Sc)
            if t - LAG >= 0:
                issue_cx(t - LAG, t)
            fill(2)
            for qc in range(QC):
                nc.scalar.activation(
                    A[:, qc * 512:(qc + 1) * 512], sc_chunks[qc][:],
                    Act.Exp, bias=0.0, scale=float(1.0 / 32.0))
            nc.vector.tensor_tensor(A[:], A[:], E[:, ki, :], op=Alu.mult)
            pend[t] = (A, CX)
            drain_tails(t)
        for t in range(NH * ST - LAG, NH * ST):
            issue_cx(t, NH * ST)
        while tail_q:
            tail_q.pop(0)[1]()

        dpool.release()
        rbp.release()
        rhp.release()
        dnp.release()
        stgp.release()
        apool.release()
        fillps.release()
        cxpsum.release()
        scpsum.release()

        # ---- epilogue: output projection ----
        prpsum = tc.alloc_tile_pool(name="prpsum", bufs=4, space="PSUM")
        opool = tc.alloc_tile_pool(name="opool", bufs=3, side="right")
        for qi in range(ST):
            for dc in range(QC):
                ps = prpsum.tile([P, 512], f32, name="pps")
                hjs = PAIRS[6:] if (qi, dc) in partials else PAIRS
                for n, hj in enumerate(hjs):
                    nc.tensor.matmul(
                        ps[:],
                        r(ctxT[:, hj, qi * P:(qi + 1) * P]),
                        r(wptiles[hj][:, dc * 512:(dc + 1) * 512]),
                        start=(n == 0),
                        stop=(n == len(hjs) - 1 and not with_bias))
                if with_bias:
                    nc.tensor.matmul(
                        ps[:],
                        r(ones_row[:, 0:P]),
                        r(bias_sb[3:4, dc * 512:(dc + 1) * 512]),
                        start=False, stop=True)
                osb = opool.tile([P, 512], f32, name="osb")
                if (qi, dc) in partials:
                    nc.vector.tensor_tensor(
                        osb[:], ps[:], partials[(qi, dc)][:], op=Alu.add)
                else:
                    nc.vector.tensor_copy(osb[:], ps[:])
                nc.sync.dma_start(
                    out_d[qi * P:(qi + 1) * P, dc * 512:(dc + 1) * 512], osb[:])
        opool.release()
        prpsum.release()
        ctxp.release()
        wpool.release()
        wkbp.release()
        wqbp.release()
        xTp.release()
        ep.release()
        vpp.release()
        ktp.release()
        qtp.release()
        const.release()

    nc.compile()
    return nc


def _get(with_bias):
    if with_bias not in _BUILT:
        _BUILT[with_bias] = _build(with_bias)
    return _BUILT[with_bias]


def _make_in_maps(inputs, with_bias):
    f = lambda a: np.ascontiguousarray(np.asarray(a), dtype=np.float32)
    x = f(inputs["x"])
    mask = f(inputs["attention_mask"])
    shared = {
        "wq": f(inputs["Wq"]), "wk": f(inputs["Wk"]),
        "wv": f(inputs["Wv"]), "wp": f(inputs["Wp"]),
    }
    if with_bias:
        shared["bq"] = f(inputs["bq"]).reshape(1, HID)
        shared["bk"] = f(inputs["bk"]).reshape(1, HID)
        shared["bv"] = f(inputs["bv"]).reshape(1, HID)
        shared["bp"] = f(inputs["bp"]).reshape(1, HID)
    return [
        dict(shared, x=x[b], mask=np.ascontiguousarray(mask[b, 0]))
        for b in range(N_CORES)
    ]


def run(trace=False, **inputs):
    from concourse.bass_utils import run_bass_kernel_spmd
    with_bias = any(
        float(np.abs(np.asarray(inputs[k])).max()) != 0.0
        for k in ("bq", "bk", "bv", "bp"))
    nc = _get(with_bias)
    in_maps = _make_in_maps(inputs, with_bias)
    res = run_bass_kernel_spmd(nc, in_maps, list(range(N_CORES)), trace=trace)
    out = np.stack([res.results[i]["out"] for i in range(N_CORES)])
    return out.astype(np.float32, copy=False), res


def kernel(**inputs):
    out, _ = run(trace=False, **inputs)
    return out


# revision 62
# speedup vs baseline: 1.0318x; 1.0075x over previous
"""Trainium2 Bass kernel for nn_MultiHeadAttention (B=8, S=1024, HID=1024, NH=16).

Strategy: data-parallel over batch — core b computes the full MHA for batch
element b (B == n_cores == 8, no collectives).

The kernel is organized to keep the PE (tensor engine) queue dense end-to-end
so the HAM clock gate stays at 2.4 GHz (scattered micro-idles re-throttle it
to 1.2 GHz, doubling every matmul):

  Prologue:  xT = x.T (PE transpose, bf16)
             E  = bf16 exp(-1e9*(mask - rowmin(mask))).T   (shared by heads)
             wq/wk/wv converted to bf16 on-chip
             QT/KT for head pair 0 (d-major, bf16);  V for ALL heads (bf16,
             s-major, +ones column per head for the softmax denominator)
  Attention: flat stream over (head, ktile); per ktile:
               S[k,q] = KT_h.T @ QT_h         (bf16, PSUM, 2x 512 chunks)
               A      = exp(S * 1/32)         (ACT, bf16)
               A     *= E[ki]                 (DVE bf16 2x mode)
               CX    += V'_h[ki].T @ A        (bf16; row 64 = denom)
             CX matmuls lag the score matmuls by 2 ktiles; QT/KT projection
             matmuls for head pair j+1 are interleaved as PE filler during
             pair j so the PE never idles.
             Per-head tail (DVE/DMA/gpsimd, off critical path): evict
             unnormalized ctx, denom -> [8,128] reshape via DRAM, DVE
             reciprocal, partition-broadcast, in-place normalize.
  Epilogue:  out = ctxT.T @ Wp (+bp)          (fp32r)

kernel() accepts the FULL inputs and returns the FULL output.
"""

import numpy as np

B, S, HID, NH = 8, 1024, 1024, 16
HD = HID // NH          # 64
P = 128                 # partitions
ST = S // P             # 8 s-tiles
HT = HID // P           # 8 hid-tiles
QC = S // 512           # 2 free-dim chunks of 512
N_CORES = 8

_BUILT = {}


def _build(with_bias):
    from concourse import bass, bacc, mybir, tile
    from concourse.masks import make_identity

    f32 = mybir.dt.float32
    f32r = mybir.dt.float32r
    bf16 = mybir.dt.bfloat16
    Alu = mybir.AluOpType
    Act = mybir.ActivationFunctionType

    nc = bacc.Bacc("TRN2", target_bir_lowering=False, debug=False,
                   num_devices=N_CORES)

    x_d = nc.declare_dram_parameter("x", [S, HID], f32, isOutput=False)
    mask_d = nc.declare_dram_parameter("mask", [S, S], f32, isOutput=False)
    wq_d = nc.declare_dram_parameter("wq", [HID, HID], f32, isOutput=False)
    wk_d = nc.declare_dram_parameter("wk", [HID, HID], f32, isOutput=False)
    wv_d = nc.declare_dram_parameter("wv", [HID, HID], f32, isOutput=False)
    wp_d = nc.declare_dram_parameter("wp", [HID, HID], f32, isOutput=False)
    if with_bias:
        bq_d = nc.declare_dram_parameter("bq", [1, HID], f32, isOutput=False)
        bk_d = nc.declare_dram_parameter("bk", [1, HID], f32, isOutput=False)
        bv_d = nc.declare_dram_parameter("bv", [1, HID], f32, isOutput=False)
        bp_d = nc.declare_dram_parameter("bp", [1, HID], f32, isOutput=False)
    out_d = nc.declare_dram_parameter("out", [S, HID], f32, isOutput=True)

    def r(ap):
        return ap.bitcast(f32r)

    with tile.TileContext(nc) as tc:
        # ---- pools (stack-ordered per side) ----
        const = tc.alloc_tile_pool(name="const", bufs=1, side="left")
        qtp = tc.alloc_tile_pool(name="qtp", bufs=1, side="left")
        ktp = tc.alloc_tile_pool(name="ktp", bufs=1, side="left")
        vpp = tc.alloc_tile_pool(name="vpp", bufs=1, side="left")
        ep = tc.alloc_tile_pool(name="ep", bufs=1, side="left")
        xTp = tc.alloc_tile_pool(name="xTp", bufs=1, side="left")
        wqbp = tc.alloc_tile_pool(name="wqbp", bufs=8, side="left")
        wkbp = tc.alloc_tile_pool(name="wkbp", bufs=8, side="left")
        # prologue-only pools (released before attention)
        wvbp = tc.alloc_tile_pool(name="wvbp", bufs=8, side="left")
        mtp = tc.alloc_tile_pool(name="mtp", bufs=8, side="left")
        minp = tc.alloc_tile_pool(name="minp", bufs=2, side="left")
        xload = tc.alloc_tile_pool(name="xload", bufs=2, side="left")
        wstg = tc.alloc_tile_pool(name="wstg", bufs=4, side="right")
        tpsum = tc.alloc_tile_pool(name="tpsum", bufs=2, space="PSUM")
        qkvpsum = tc.alloc_tile_pool(name="qkvpsum", bufs=4, space="PSUM")

        ident = const.tile([P, P], f32)
        make_identity(nc, ident)
        if with_bias:
            ones_row = const.tile([1, 512], f32r)
            nc.vector.memset(ones_row[:], 1.0)
            ones_bf = const.tile([1, 512], bf16)
            nc.vector.memset(ones_bf[:], 1.0)
            bias_sb = const.tile([4, HID], f32r)
            nc.sync.dma_start(bias_sb[0:1, :], bq_d[:].bitcast(f32r))
            nc.sync.dma_start(bias_sb[1:2, :], bk_d[:].bitcast(f32r))
            nc.sync.dma_start(bias_sb[2:3, :], bv_d[:].bitcast(f32r))
            nc.sync.dma_start(bias_sb[3:4, :], bp_d[:].bitcast(f32r))
            bias_bf = const.tile([4, HID], bf16)
            nc.scalar.copy(bias_bf[:], bias_sb[:].bitcast(f32))

        QT = qtp.tile([P, HT, S], bf16)              # QT[p, j, s] = Q[s, j*128+p]
        KT = ktp.tile([P, HT, S], bf16)
        Vp = vpp.tile([P, ST, NH, HD + 1], bf16)     # V'[p, si, h, c]
        E = ep.tile([P, ST, S], bf16)                # E[p, ki, q] = exp-mask
        xT = xTp.tile([P, HT, S], bf16)              # xT[p, j, s] = x[s, j*128+p]

        nc.vector.memset(Vp[:, :, :, HD:HD + 1], 1.0)

        # ---- prologue: load x, transpose to xT (bf16) ----
        for si in range(ST):
            xs = xload.tile([P, HID], f32, name="xs")
            # alternate the two HWDGE engines (SP / Activation) so x tiles
            # arrive two-at-a-time instead of pacing the transposes
            dma_eng = nc.sync if si % 2 == 0 else nc.scalar
            dma_eng.dma_start(xs[:], x_d[si * P:(si + 1) * P, :])
            for g in range(2):  # groups of 4 hid-tiles
                tp = tpsum.tile([P, 512], f32, name="tp")
                for u in range(4):
                    hj = g * 4 + u
                    nc.tensor.transpose(tp[:, u * P:(u + 1) * P],
                                        xs[:, hj * P:(hj + 1) * P], ident[:])
                nc.scalar.copy(
                    xT[:, g * 4:(g + 1) * 4, si * P:(si + 1) * P],
                    tp[:].rearrange("p (a b) -> p a b", a=4))

        # ---- weights: load f32, convert to bf16 on-chip ----
        def load_w_bf(dram, pool, dma_eng):
            tiles = []
            for kj in range(HT):
                ws = wstg.tile([P, HID], f32, name="ws")
                dma_eng.dma_start(ws[:], dram[kj * P:(kj + 1) * P, :])
                wb = pool.tile([P, HID], bf16, name="wb")
                nc.vector.tensor_copy(wb[:], ws[:])
                tiles.append(wb)
            return tiles

        # ---- mask -> E: DMA + rowmin/scale (DVE), transpose (PE), exp (ACT)
        # (mask DMAs and DVE prep run BEFORE the weight converts so the PE's
        # mask transposes aren't gated behind 24 convert ops in the DVE queue)
        mts = []
        for qi in range(ST):
            mt = mtp.tile([P, S], f32, name="mt")
            nc.scalar.dma_start(mt[:], mask_d[qi * P:(qi + 1) * P, :])
            mn = minp.tile([P, 1], f32, name="mn")
            nc.vector.tensor_reduce(mn[:], mt[:], axis=mybir.AxisListType.X,
                                    op=Alu.min)
            # mt = (mask - rowmin) * (-1e9)
            nc.vector.tensor_scalar(mt[:], mt[:], mn[:], -1.0e9,
                                    op0=Alu.subtract, op1=Alu.mult)
            mts.append(mt)

        wqb = load_w_bf(wq_d, wqbp, nc.sync)
        wkb = load_w_bf(wk_d, wkbp, nc.scalar)
        wvb = load_w_bf(wv_d, wvbp, nc.sync)

        for qi in range(ST):
            mt = mts[qi]
            for g in range(2):
                tp = tpsum.tile([P, 512], f32, name="tp")
                for u in range(4):
                    ki = g * 4 + u
                    nc.tensor.transpose(tp[:, u * P:(u + 1) * P],
                                        mt[:, ki * P:(ki + 1) * P], ident[:])
                nc.scalar.activation(
                    E[:, g * 4:(g + 1) * 4, qi * P:(qi + 1) * P],
                    tp[:].rearrange("p (a b) -> p a b", a=4),
                    Act.Exp, bias=0.0, scale=1.0)

        # ---- prologue projections: QT/KT for head pair 0, V for all ----
        def qk_group(dst, wtiles, brow, dj, sc, pool):
            ps = pool.tile([P, 512], f32, name="ps")
            for kj in range(HT):
                nc.tensor.matmul(
                    ps[:],
                    wtiles[kj][:, dj * P:(dj + 1) * P],
                    xT[:, kj, sc * 512:(sc + 1) * 512],
                    start=(kj == 0), stop=(kj == HT - 1 and not with_bias))
            if with_bias:
                nc.tensor.matmul(
                    ps[:],
                    bias_bf[brow:brow + 1, dj * P:(dj + 1) * P],
                    ones_bf[:],
                    start=False, stop=True)
            nc.vector.tensor_copy(dst[:, dj, sc * 512:(sc + 1) * 512], ps[:])

        PAIRS = [1, 2, 3, 4, 5, 6, 7, 0]  # pair processing order; pair 0
        # last so the epilogue's hj=0 contraction step is the only one
        # gated on the final head's normalize
        for sc in range(QC):
            qk_group(QT, wqb, 0, PAIRS[0], sc, qkvpsum)
            qk_group(KT, wkb, 1, PAIRS[0], sc, qkvpsum)

        for si in range(ST):
            for dc in range(QC):
                ps = qkvpsum.tile([P, 512], f32, name="ps")
                for kj in range(HT):
                    nc.tensor.matmul(
                        ps[:],
                        xT[:, kj, si * P:(si + 1) * P],
                        wvb[kj][:, dc * 512:(dc + 1) * 512],
                        start=(kj == 0), stop=(kj == HT - 1 and not with_bias))
                if with_bias:
                    nc.tensor.matmul(
                        ps[:],
                        ones_bf[:, 0:P],
                        bias_bf[2:3, dc * 512:(dc + 1) * 512],
                        start=False, stop=True)
                nc.vector.tensor_copy(
                    Vp[:, si, dc * 8:(dc + 1) * 8, 0:HD],
                    ps[:].rearrange("p (h c) -> p h c", h=8))

        qkvpsum.release()
        tpsum.release()
        wstg.release()
        xload.release()
        minp.release()
        mtp.release()
        wvbp.release()

        # ---- attention + interleaved QT/KT filler ----
        wpool = tc.alloc_tile_pool(name="wpool", bufs=8, side="right")
        ctxp = tc.alloc_tile_pool(name="ctxp", bufs=1, side="right")
        scpsum = tc.alloc_tile_pool(name="scpsum", bufs=3, space="PSUM")
        cxpsum = tc.alloc_tile_pool(name="cxpsum", bufs=2, space="PSUM")
        fillps = tc.alloc_tile_pool(name="fillps", bufs=1, space="PSUM")
        apool = tc.alloc_tile_pool(name="apool", bufs=4, side="right")
        stgp = tc.alloc_tile_pool(name="stgp", bufs=1, side="right")
        dnp = tc.alloc_tile_pool(name="dnp", bufs=2, side="right")
        rhp = tc.alloc_tile_pool(name="rhp", bufs=1, side="right")
        rbp = tc.alloc_tile_pool(name="rbp", bufs=2, side="right")
        dpool = tc.alloc_tile_pool(name="dpool", bufs=1, space="DRAM")

        ctxT = ctxp.tile([P, HT, S], f32r)           # ctxT[p, j, q]
        dscr = dpool.tile([1, NH * S], f32, name="dscr")
        dscr2 = dpool.tile([1, NH * S], f32, name="dscr2")

        # dummy broadcast: pre-load the gpsimd partition_broadcast library
        # now, not in the first head's normalize chain (LIBRARY_RELOAD there
        # stalled the DVE FIFO long enough to re-throttle the HAM clock)
        rh0 = rhp.tile([1, S], f32, name="rh")
        nc.vector.memset(rh0[:], 1.0)
        rb0 = rbp.tile([P, S], f32, name="RB")
        nc.gpsimd.partition_broadcast(rb0[:], rh0[:])

        # filler: QT/KT projections for head pair j+1, one closure per MM
        def mk_fill(dst, wtiles, brow, dj, sc, kj, holder):
            def go():
                if kj == 0:
                    holder["ps"] = fillps.tile([P, 512], f32, name="fps")
                ps = holder["ps"]
                nc.tensor.matmul(
                    ps[:],
                    wtiles[kj][:, dj * P:(dj + 1) * P],
                    xT[:, kj, sc * 512:(sc + 1) * 512],
                    start=(kj == 0), stop=(kj == HT - 1 and not with_bias))
                if kj == HT - 1:
                    if with_bias:
                        nc.tensor.matmul(
                            ps[:],
                            bias_bf[brow:brow + 1, dj * P:(dj + 1) * P],
                            ones_bf[:],
                            start=False, stop=True)
                    nc.scalar.copy(dst[:, dj, sc * 512:(sc + 1) * 512], ps[:])
            return go

        flat = []
        bstart = [0] * 8  # batch b -> start index in flat
        for b in range(7):
            bstart[b] = len(flat)
            for dst, wt, brow in ((QT, wqb, 0), (KT, wkb, 1)):
                for sc in range(QC):
                    holder = {}
                    for kj in range(HT):
                        flat.append(mk_fill(dst, wt, brow, PAIRS[b + 1], sc,
                                            kj, holder))
        bstart[7] = len(flat)
        fi = [0]

        def fill(n):
            while n > 0 and fi[0] < len(flat):
                flat[fi[0]]()
                fi[0] += 1
                n -= 1

        def flush(j):
            end = bstart[j + 1] if j + 1 < len(bstart) else len(flat)
            while fi[0] < end:
                flat[fi[0]]()
                fi[0] += 1

        LAG = 2  # ktiles the CX matmuls trail the score matmuls by

        # Per-head tail steps run as deferred closures with due-times. The
        # normalize (c4) waits on the whole reciprocal/broadcast chain; if it
        # enters the DVE FIFO too early it blocks every queued A-multiply
        # behind it, starving the CX matmuls and re-throttling the HAM clock.
        tail_q = []  # (due_t, closure), appended in due order
        tail_state = {}

        def drain_tails(t):
            while tail_q and tail_q[0][0] <= t:
                tail_q.pop(0)[1]()

        def head_tail(h, CX, t_now):
            j, po = h // 2, (h % 2) * 64

            def c1():  # denom row -> SBUF -> DRAM -> [8,128]
                stg = stgp.tile([1, S], f32, name="stg")
                nc.vector.tensor_copy(stg[:], CX[HD:HD + 1, :])
                nc.sync.dma_start(dscr[0:1, h * S:(h + 1) * S], stg[:])
                dn = dnp.tile([8, P], f32, name="dn")
                nc.sync.dma_start(
                    dn[:],
                    dscr[0:1, h * S:(h + 1) * S].rearrange(
                        "p (a b) -> (p a) b", a=8))
                tail_state[h] = dn

            def c2():  # evict unnormalized ctx (frees the CX PSUM bank)
                nc.vector.tensor_copy(ctxT[po:po + 64, j, :], CX[0:HD, :])

            def c3():  # reciprocal + reshape back + broadcast
                dn = tail_state[h]
                nc.vector.reciprocal(dn[:], dn[:])
                nc.sync.dma_start(
                    dscr2[0:1, h * S:(h + 1) * S].rearrange(
                        "p (a b) -> (p a) b", a=8),
                    dn[:])
                rh = rhp.tile([1, S], f32, name="rh")
                nc.sync.dma_start(rh[:], dscr2[0:1, h * S:(h + 1) * S])
                RB = rbp.tile([P, S], f32, name="RB")
                nc.gpsimd.partition_broadcast(RB[:], rh[:])
                tail_state[h] = RB

            def c4():  # normalize in place (chain has finished by now)
                RB = tail_state.pop(h)
                nc.vector.tensor_tensor(
                    ctxT[po:po + 64, j, :],
                    ctxT[po:po + 64, j, :].bitcast(f32),
                    RB[po:po + 64, :],
                    op=Alu.mult)

            tail_q.extend([(t_now + 1, c1), (t_now + 2, c2),
                           (t_now + 3, c3), (t_now + 7, c4)])

        def head_of(t):
            return 2 * PAIRS[t // 16] + (t // ST) % 2

        def issue_cx(t, t_now):
            h, ki = head_of(t), t % ST
            A, CX = pend[t]
            for qc in range(QC):
                nc.tensor.matmul(
                    CX[0:HD + 1, qc * 512:(qc + 1) * 512],
                    Vp[:, ki, h, :],
                    A[:, qc * 512:(qc + 1) * 512],
                    start=(ki == 0), stop=(ki == ST - 1))
            if ki == ST - 1:
                head_tail(h, CX, t_now)

        pend = {}
        partials = {}
        CX = None
        for t in range(NH * ST):  # flat (head, ktile) stream
            h, ki = head_of(t), t % ST
            j, po = h // 2, (h % 2) * 64
            if t % 16 == 0 and t > 0:
                flush(t // 16 - 1)  # QT/KT for this pair must be complete
            if t == 112:
                # all filler done; free the pools the filler was reading and
                # make room for partial output-projection accumulators
                wkbp.release()
                wqbp.release()
                xTp.release()
                o2pool = tc.alloc_tile_pool(name="o2pool", bufs=8,
                                            side="right")
            if ki == 0:
                CX = cxpsum.tile([P, S], f32, name="CX")
            if t == 80:  # prefetch Wp once filler pressure eases
                wptiles = [wpool.tile([P, HID], f32r, name="wt")
                           for _ in range(HT)]
                for kj in range(HT):
                    nc.sync.dma_start(
                        wptiles[kj][:],
                        wp_d[kj * P:(kj + 1) * P, :].bitcast(f32r))
            A = apool.tile([P, S], bf16, name="A")
            sc_chunks = []
            for qc in range(QC):
                Sc = scpsum.tile([P, 512], f32, name="Sc")
                nc.tensor.matmul(
                    Sc[:],
                    KT[po:po + 64, j, ki * P:(ki + 1) * P],
                    QT[po:po + 64, j, qc * 512:(qc + 1) * 512],
                    start=True, stop=True)
                sc_chunks.append(Sc)
            if t - LAG >= 0:
                issue_cx(t - LAG, t)
            fill(2)
            # last pair has no projection filler left: keep the PE warm with
            # partial epilogue chunks over the six long-finished head pairs
            if t >= 112 and (t - 112) % 2 == 0:
                qi = (t - 112) // 2
                ps = scpsum.tile([P, 512], f32, name="Sc")
                for m, hj in enumerate(PAIRS[:6]):
                    nc.tensor.matmul(
                        ps[:],
                        r(ctxT[:, hj, qi * P:(qi + 1) * P]),
                        r(wptiles[hj][:, 0:512]),
                        start=(m == 0), stop=(m == 5))
                po2 = o2pool.tile([P, 512], f32, name="po2")
                nc.vector.tensor_copy(po2[:], ps[:])
                partials[(qi, 0)] = po2
            for qc in range(QC):
                nc.scalar.activation(
                    A[:, qc * 512:(qc + 1) * 512], sc_chunks[qc][:],
                    Act.Exp, bias=0.0, scale=float(1.0 / 32.0))
            nc.vector.tensor_tensor(A[:], A[:], E[:, ki, :], op=Alu.mult)
            pend[t] = (A, CX)
            drain_tails(t)
        for t in range(NH * ST - LAG, NH * ST):
            issue_cx(t, NH * ST)
        while tail_q:
            tail_q.pop(0)[1]()

        dpool.release()
        fillps.release()
        cxpsum.release()
        scpsum.release()

        # ---- epilogue: output projection ----
        prpsum = tc.alloc_tile_pool(name="prpsum", bufs=4, space="PSUM")
        opool = tc.alloc_tile_pool(name="opool", bufs=3, side="right")
        for qi in range(ST):
            for dc in range(QC):
                ps = prpsum.tile([P, 512], f32, name="pps")
                hjs = PAIRS[6:] if (qi, dc) in partials else PAIRS
                for n, hj in enumerate(hjs):
                    nc.tensor.matmul(
                        ps[:],
                        r(ctxT[:, hj, qi * P:(qi + 1) * P]),
                        r(wptiles[hj][:, dc * 512:(dc + 1) * 512]),
                        start=(n == 0),
                        stop=(n == len(hjs) - 1 and not with_bias))
                if with_bias:
                    nc.tensor.matmul(
                        ps[:],
                        r(ones_row[:, 0:P]),
                        r(bias_sb[3:4, dc * 512:(dc + 1) * 512]),
                        start=False, stop=True)
                osb = opool.tile([P, 512], f32, name="osb")
                if (qi, dc) in partials:
                    nc.vector.tensor_tensor(
                        osb[:], ps[:], partials[(qi, dc)][:], op=Alu.add)
                else:
                    nc.vector.tensor_copy(osb[:], ps[:])
                nc.sync.dma_start(
                    out_d[qi * P:(qi + 1) * P, dc * 512:(dc + 1) * 512], osb[:])
        opool.release()
        prpsum.release()
        o2pool.release()
        rbp.release()
        rhp.release()
        dnp.release()
        stgp.release()
        apool.release()
        ctxp.release()
        wpool.release()
        ep.release()
        vpp.release()
        ktp.release()
        qtp.release()
        const.release()

    nc.compile()
    return nc


def _get(with_bias):
    if with_bias not in _BUILT:
        _BUILT[with_bias] = _build(with_bias)
    return _BUILT[with_bias]


def _make_in_maps(inputs, with_bias):
    f = lambda a: np.ascontiguousarray(np.asarray(a), dtype=np.float32)
    x = f(inputs["x"])
    mask = f(inputs["attention_mask"])
    shared = {
        "wq": f(inputs["Wq"]), "wk": f(inputs["Wk"]),
        "wv": f(inputs["Wv"]), "wp": f(inputs["Wp"]),
    }
    if with_bias:
        shared["bq"] = f(inputs["bq"]).reshape(1, HID)
        shared["bk"] = f(inputs["bk"]).reshape(1, HID)
        shared["bv"] = f(inputs["bv"]).reshape(1, HID)
        shared["bp"] = f(inputs["bp"]).reshape(1, HID)
    return [
        dict(shared, x=x[b], mask=np.ascontiguousarray(mask[b, 0]))
        for b in range(N_CORES)
    ]


def run(trace=False, **inputs):
    from concourse.bass_utils import run_bass_kernel_spmd
    with_bias = any(
        float(np.abs(np.asarray(inputs[k])).max()) != 0.0
        for k in ("bq", "bk", "bv", "bp"))
    nc = _get(with_bias)
    in_maps = _make_in_maps(inputs, with_bias)
    res = run_bass_kernel_spmd(nc, in_maps, list(range(N_CORES)), trace=trace)
    out = np.stack([res.results[i]["out"] for i in range(N_CORES)])
    return out.astype(np.float32, copy=False), res


def kernel(**inputs):
    out, _ = run(trace=False, **inputs)
    return out


# revision 64
# speedup vs baseline: 1.0549x; 1.0224x over previous
"""Trainium2 Bass kernel for nn_MultiHeadAttention (B=8, S=1024, HID=1024, NH=16).

Strategy: data-parallel over batch — core b computes the full MHA for batch
element b (B == n_cores == 8, no collectives).

The kernel is organized to keep the PE (tensor engine) queue dense end-to-end
so the HAM clock gate stays at 2.4 GHz (scattered micro-idles re-throttle it
to 1.2 GHz, doubling every matmul):

  Prologue:  xT = x.T (PE transpose, bf16)
             E  = bf16 exp(-1e9*(mask - rowmin(mask))).T   (shared by heads)
             wq/wk/wv converted to bf16 on-chip
             QT/KT for head pair 0 (d-major, bf16);  V for ALL heads (bf16,
             s-major, +ones column per head for the softmax denominator)
  Attention: flat stream over (head, ktile); per ktile:
               S[k,q] = KT_h.T @ QT_h         (bf16, PSUM, 2x 512 chunks)
               A      = exp(S * 1/32)         (ACT, bf16)
               A     *= E[ki]                 (DVE bf16 2x mode)
               CX    += V'_h[ki].T @ A        (bf16; row 64 = denom)
             CX matmuls lag the score matmuls by 2 ktiles; QT/KT projection
             matmuls for head pair j+1 are interleaved as PE filler during
             pair j so the PE never idles.
             Per-head tail (DVE/DMA/gpsimd, off critical path): evict
             unnormalized ctx, denom -> [8,128] reshape via DRAM, DVE
             reciprocal, partition-broadcast, in-place normalize.
  Epilogue:  out = ctxT.T @ Wp (+bp)          (fp32r)

kernel() accepts the FULL inputs and returns the FULL output.
"""

import numpy as np

B, S, HID, NH = 8, 1024, 1024, 16
HD = HID // NH          # 64
P = 128                 # partitions
ST = S // P             # 8 s-tiles
HT = HID // P           # 8 hid-tiles
QC = S // 512           # 2 free-dim chunks of 512
N_CORES = 8

_BUILT = {}


def _build(with_bias):
    from concourse import bass, bacc, mybir, tile
    from concourse.masks import make_identity

    f32 = mybir.dt.float32
    f32r = mybir.dt.float32r
    bf16 = mybir.dt.bfloat16
    Alu = mybir.AluOpType
    Act = mybir.ActivationFunctionType

    nc = bacc.Bacc("TRN2", target_bir_lowering=False, debug=False,
                   num_devices=N_CORES)

    x_d = nc.declare_dram_parameter("x", [S, HID], f32, isOutput=False)
    mask_d = nc.declare_dram_parameter("mask", [S, S], f32, isOutput=False)
    wq_d = nc.declare_dram_parameter("wq", [HID, HID], f32, isOutput=False)
    wk_d = nc.declare_dram_parameter("wk", [HID, HID], f32, isOutput=False)
    wv_d = nc.declare_dram_parameter("wv", [HID, HID], f32, isOutput=False)
    wp_d = nc.declare_dram_parameter("wp", [HID, HID], f32, isOutput=False)
    if with_bias:
        bq_d = nc.declare_dram_parameter("bq", [1, HID], f32, isOutput=False)
        bk_d = nc.declare_dram_parameter("bk", [1, HID], f32, isOutput=False)
        bv_d = nc.declare_dram_parameter("bv", [1, HID], f32, isOutput=False)
        bp_d = nc.declare_dram_parameter("bp", [1, HID], f32, isOutput=False)
    out_d = nc.declare_dram_parameter("out", [S, HID], f32, isOutput=True)

    def r(ap):
        return ap.bitcast(f32r)

    with tile.TileContext(nc) as tc:
        # ---- pools (stack-ordered per side) ----
        const = tc.alloc_tile_pool(name="const", bufs=1, side="left")
        qtp = tc.alloc_tile_pool(name="qtp", bufs=1, side="left")
        ktp = tc.alloc_tile_pool(name="ktp", bufs=1, side="left")
        vpp = tc.alloc_tile_pool(name="vpp", bufs=1, side="left")
        ep = tc.alloc_tile_pool(name="ep", bufs=1, side="left")
        xTp = tc.alloc_tile_pool(name="xTp", bufs=1, side="left")
        wqbp = tc.alloc_tile_pool(name="wqbp", bufs=8, side="left")
        wkbp = tc.alloc_tile_pool(name="wkbp", bufs=8, side="left")
        # prologue-only pools (released before attention)
        wvbp = tc.alloc_tile_pool(name="wvbp", bufs=8, side="left")
        mtp = tc.alloc_tile_pool(name="mtp", bufs=8, side="left")
        minp = tc.alloc_tile_pool(name="minp", bufs=2, side="left")
        xload = tc.alloc_tile_pool(name="xload", bufs=2, side="left")
        wstg = tc.alloc_tile_pool(name="wstg", bufs=4, side="right")
        tpsum = tc.alloc_tile_pool(name="tpsum", bufs=2, space="PSUM")
        qkvpsum = tc.alloc_tile_pool(name="qkvpsum", bufs=4, space="PSUM")

        ident = const.tile([P, P], f32)
        make_identity(nc, ident)
        if with_bias:
            ones_row = const.tile([1, 512], f32r)
            nc.vector.memset(ones_row[:], 1.0)
            ones_bf = const.tile([1, 512], bf16)
            nc.vector.memset(ones_bf[:], 1.0)
            bias_sb = const.tile([4, HID], f32r)
            nc.sync.dma_start(bias_sb[0:1, :], bq_d[:].bitcast(f32r))
            nc.sync.dma_start(bias_sb[1:2, :], bk_d[:].bitcast(f32r))
            nc.sync.dma_start(bias_sb[2:3, :], bv_d[:].bitcast(f32r))
            nc.sync.dma_start(bias_sb[3:4, :], bp_d[:].bitcast(f32r))
            bias_bf = const.tile([4, HID], bf16)
            nc.scalar.copy(bias_bf[:], bias_sb[:].bitcast(f32))

        QT = qtp.tile([P, HT, S], bf16)              # QT[p, j, s] = Q[s, j*128+p]
        KT = ktp.tile([P, HT, S], bf16)
        Vp = vpp.tile([P, ST, NH, HD + 1], bf16)     # V'[p, si, h, c]
        E = ep.tile([P, ST, S], bf16)                # E[p, ki, q] = exp-mask
        xT = xTp.tile([P, HT, S], bf16)              # xT[p, j, s] = x[s, j*128+p]

        nc.vector.memset(Vp[:, :, :, HD:HD + 1], 1.0)

        # ---- prologue: load x, transpose to xT (bf16) ----
        for si in range(ST):
            xs = xload.tile([P, HID], f32, name="xs")
            # alternate the two HWDGE engines (SP / Activation) so x tiles
            # arrive two-at-a-time instead of pacing the transposes
            dma_eng = nc.sync if si % 2 == 0 else nc.scalar
            dma_eng.dma_start(xs[:], x_d[si * P:(si + 1) * P, :])
            for g in range(2):  # groups of 4 hid-tiles
                tp = tpsum.tile([P, 512], f32, name="tp")
                for u in range(4):
                    hj = g * 4 + u
                    nc.tensor.transpose(tp[:, u * P:(u + 1) * P],
                                        xs[:, hj * P:(hj + 1) * P], ident[:])
                nc.scalar.copy(
                    xT[:, g * 4:(g + 1) * 4, si * P:(si + 1) * P],
                    tp[:].rearrange("p (a b) -> p a b", a=4))

        # ---- weights: load f32, convert to bf16 on-chip ----
        def load_w_bf(dram, pool, dma_eng, cvt_scalar=False):
            tiles = []
            for kj in range(HT):
                ws = wstg.tile([P, HID], f32, name="ws")
                dma_eng.dma_start(ws[:], dram[kj * P:(kj + 1) * P, :])
                wb = pool.tile([P, HID], bf16, name="wb")
                if cvt_scalar:
                    nc.scalar.copy(wb[:], ws[:])
                else:
                    nc.vector.tensor_copy(wb[:], ws[:])
                tiles.append(wb)
            return tiles

        # ---- mask -> E: DMA + rowmin/scale (DVE), transpose (PE), exp (ACT)
        # (mask DMAs and DVE prep run BEFORE the weight converts so the PE's
        # mask transposes aren't gated behind 24 convert ops in the DVE queue)
        mts = []
        for qi in range(ST):
            mt = mtp.tile([P, S], f32, name="mt")
            nc.scalar.dma_start(mt[:], mask_d[qi * P:(qi + 1) * P, :])
            mn = minp.tile([P, 1], f32, name="mn")
            nc.vector.tensor_reduce(mn[:], mt[:], axis=mybir.AxisListType.X,
                                    op=Alu.min)
            # mt = (mask - rowmin) * (-1e9)
            nc.vector.tensor_scalar(mt[:], mt[:], mn[:], -1.0e9,
                                    op0=Alu.subtract, op1=Alu.mult)
            mts.append(mt)

        wqb = load_w_bf(wq_d, wqbp, nc.sync)
        wkb = load_w_bf(wk_d, wkbp, nc.scalar)
        # wv converts on the scalar engine: the DVE is the prologue straggler
        # (mask prep + wq/wk converts), while scalar is idle after xT evicts
        wvb = load_w_bf(wv_d, wvbp, nc.sync, cvt_scalar=True)

        for qi in range(ST):
            mt = mts[qi]
            for g in range(2):
                tp = tpsum.tile([P, 512], f32, name="tp")
                for u in range(4):
                    ki = g * 4 + u
                    nc.tensor.transpose(tp[:, u * P:(u + 1) * P],
                                        mt[:, ki * P:(ki + 1) * P], ident[:])
                nc.scalar.activation(
                    E[:, g * 4:(g + 1) * 4, qi * P:(qi + 1) * P],
                    tp[:].rearrange("p (a b) -> p a b", a=4),
                    Act.Exp, bias=0.0, scale=1.0)

        # ---- prologue projections: QT/KT for head pair 0, V for all ----
        def qk_group(dst, wtiles, brow, dj, sc, pool):
            ps = pool.tile([P, 512], f32, name="ps")
            for kj in range(HT):
                nc.tensor.matmul(
                    ps[:],
                    wtiles[kj][:, dj * P:(dj + 1) * P],
                    xT[:, kj, sc * 512:(sc + 1) * 512],
                    start=(kj == 0), stop=(kj == HT - 1 and not with_bias))
            if with_bias:
                nc.tensor.matmul(
                    ps[:],
                    bias_bf[brow:brow + 1, dj * P:(dj + 1) * P],
                    ones_bf[:],
                    start=False, stop=True)
            nc.vector.tensor_copy(dst[:, dj, sc * 512:(sc + 1) * 512], ps[:])

        PAIRS = [1, 2, 3, 4, 5, 6, 7, 0]  # pair processing order; pair 0
        # last so the epilogue's hj=0 contraction step is the only one
        # gated on the final head's normalize
        for sc in range(QC):
            qk_group(QT, wqb, 0, PAIRS[0], sc, qkvpsum)
            qk_group(KT, wkb, 1, PAIRS[0], sc, qkvpsum)

        for si in range(ST):
            for dc in range(QC):
                ps = qkvpsum.tile([P, 512], f32, name="ps")
                for kj in range(HT):
                    nc.tensor.matmul(
                        ps[:],
                        xT[:, kj, si * P:(si + 1) * P],
                        wvb[kj][:, dc * 512:(dc + 1) * 512],
                        start=(kj == 0), stop=(kj == HT - 1 and not with_bias))
                if with_bias:
                    nc.tensor.matmul(
                        ps[:],
                        ones_bf[:, 0:P],
                        bias_bf[2:3, dc * 512:(dc + 1) * 512],
                        start=False, stop=True)
                nc.vector.tensor_copy(
                    Vp[:, si, dc * 8:(dc + 1) * 8, 0:HD],
                    ps[:].rearrange("p (h c) -> p h c", h=8))

        qkvpsum.release()
        tpsum.release()
        wstg.release()
        xload.release()
        minp.release()
        mtp.release()
        wvbp.release()

        # ---- attention + interleaved QT/KT filler ----
        wpool = tc.alloc_tile_pool(name="wpool", bufs=8, side="right")
        ctxp = tc.alloc_tile_pool(name="ctxp", bufs=1, side="right")
        scpsum = tc.alloc_tile_pool(name="scpsum", bufs=3, space="PSUM")
        cxpsum = tc.alloc_tile_pool(name="cxpsum", bufs=2, space="PSUM")
        fillps = tc.alloc_tile_pool(name="fillps", bufs=1, space="PSUM")
        apool = tc.alloc_tile_pool(name="apool", bufs=4, side="right")
        stgp = tc.alloc_tile_pool(name="stgp", bufs=1, side="right")
        dnp = tc.alloc_tile_pool(name="dnp", bufs=2, side="right")
        rhp = tc.alloc_tile_pool(name="rhp", bufs=1, side="right")
        rbp = tc.alloc_tile_pool(name="rbp", bufs=2, side="right")
        dpool = tc.alloc_tile_pool(name="dpool", bufs=1, space="DRAM")

        ctxT = ctxp.tile([P, HT, S], f32r)           # ctxT[p, j, q]
        dscr = dpool.tile([1, NH * S], f32, name="dscr")
        dscr2 = dpool.tile([1, NH * S], f32, name="dscr2")

        # dummy broadcast: pre-load the gpsimd partition_broadcast library
        # now, not in the first head's normalize chain (LIBRARY_RELOAD there
        # stalled the DVE FIFO long enough to re-throttle the HAM clock)
        rh0 = rhp.tile([1, S], f32, name="rh")
        nc.vector.memset(rh0[:], 1.0)
        rb0 = rbp.tile([P, S], f32, name="RB")
        nc.gpsimd.partition_broadcast(rb0[:], rh0[:])

        # filler: QT/KT projections for head pair j+1, one closure per MM
        def mk_fill(dst, wtiles, brow, dj, sc, kj, holder):
            def go():
                if kj == 0:
                    holder["ps"] = fillps.tile([P, 512], f32, name="fps")
                ps = holder["ps"]
                nc.tensor.matmul(
                    ps[:],
                    wtiles[kj][:, dj * P:(dj + 1) * P],
                    xT[:, kj, sc * 512:(sc + 1) * 512],
                    start=(kj == 0), stop=(kj == HT - 1 and not with_bias))
                if kj == HT - 1:
                    if with_bias:
                        nc.tensor.matmul(
                            ps[:],
                            bias_bf[brow:brow + 1, dj * P:(dj + 1) * P],
                            ones_bf[:],
                            start=False, stop=True)
                    nc.scalar.copy(dst[:, dj, sc * 512:(sc + 1) * 512], ps[:])
            return go

        flat = []
        bstart = [0] * 8  # batch b -> start index in flat
        for b in range(7):
            bstart[b] = len(flat)
            for dst, wt, brow in ((QT, wqb, 0), (KT, wkb, 1)):
                for sc in range(QC):
                    holder = {}
                    for kj in range(HT):
                        flat.append(mk_fill(dst, wt, brow, PAIRS[b + 1], sc,
                                            kj, holder))
        bstart[7] = len(flat)
        fi = [0]

        def fill(n):
            while n > 0 and fi[0] < len(flat):
                flat[fi[0]]()
                fi[0] += 1
                n -= 1

        def flush(j):
            end = bstart[j + 1] if j + 1 < len(bstart) else len(flat)
            while fi[0] < end:
                flat[fi[0]]()
                fi[0] += 1

        LAG = 2  # ktiles the CX matmuls trail the score matmuls by

        # Per-head tail steps run as deferred closures with due-times. The
        # normalize (c4) waits on the whole reciprocal/broadcast chain; if it
        # enters the DVE FIFO too early it blocks every queued A-multiply
        # behind it, starving the CX matmuls and re-throttling the HAM clock.
        tail_q = []  # (due_t, closure), appended in due order
        tail_state = {}

        def drain_tails(t):
            while tail_q and tail_q[0][0] <= t:
                tail_q.pop(0)[1]()

        def head_tail(h, CX, t_now):
            j, po = h // 2, (h % 2) * 64

            def c1():  # denom row -> SBUF -> DRAM -> [8,128]
                stg = stgp.tile([1, S], f32, name="stg")
                nc.vector.tensor_copy(stg[:], CX[HD:HD + 1, :])
                nc.sync.dma_start(dscr[0:1, h * S:(h + 1) * S], stg[:])
                dn = dnp.tile([8, P], f32, name="dn")
                nc.sync.dma_start(
                    dn[:],
                    dscr[0:1, h * S:(h + 1) * S].rearrange(
                        "p (a b) -> (p a) b", a=8))
                tail_state[h] = dn

            def c2():  # evict unnormalized ctx (frees the CX PSUM bank)
                nc.vector.tensor_copy(ctxT[po:po + 64, j, :], CX[0:HD, :])

            def c3():  # reciprocal + reshape back + broadcast
                dn = tail_state[h]
                nc.vector.reciprocal(dn[:], dn[:])
                nc.sync.dma_start(
                    dscr2[0:1, h * S:(h + 1) * S].rearrange(
                        "p (a b) -> (p a) b", a=8),
                    dn[:])
                rh = rhp.tile([1, S], f32, name="rh")
                nc.sync.dma_start(rh[:], dscr2[0:1, h * S:(h + 1) * S])
                RB = rbp.tile([P, S], f32, name="RB")
                nc.gpsimd.partition_broadcast(RB[:], rh[:])
                tail_state[h] = RB

            def c4():  # normalize in place (chain has finished by now)
                RB = tail_state.pop(h)
                nc.vector.tensor_tensor(
                    ctxT[po:po + 64, j, :],
                    ctxT[po:po + 64, j, :].bitcast(f32),
                    RB[po:po + 64, :],
                    op=Alu.mult)

            tail_q.extend([(t_now + 1, c1), (t_now + 2, c2),
                           (t_now + 3, c3), (t_now + 7, c4)])

        def head_of(t):
            return 2 * PAIRS[t // 16] + (t // ST) % 2

        def issue_cx(t, t_now):
            h, ki = head_of(t), t % ST
            A, CX = pend[t]
            for qc in range(QC):
                nc.tensor.matmul(
                    CX[0:HD + 1, qc * 512:(qc + 1) * 512],
                    Vp[:, ki, h, :],
                    A[:, qc * 512:(qc + 1) * 512],
                    start=(ki == 0), stop=(ki == ST - 1))
            if ki == ST - 1:
                head_tail(h, CX, t_now)

        pend = {}
        partials = {}
        CX = None
        for t in range(NH * ST):  # flat (head, ktile) stream
            h, ki = head_of(t), t % ST
            j, po = h // 2, (h % 2) * 64
            if t % 16 == 0 and t > 0:
                flush(t // 16 - 1)  # QT/KT for this pair must be complete
            if t == 112:
                # all filler done; free the pools the filler was reading and
                # make room for partial output-projection accumulators
                wkbp.release()
                wqbp.release()
                xTp.release()
                o2pool = tc.alloc_tile_pool(name="o2pool", bufs=8,
                                            side="right")
            if ki == 0:
                CX = cxpsum.tile([P, S], f32, name="CX")
            if t == 80:  # prefetch Wp once filler pressure eases
                wptiles = [wpool.tile([P, HID], f32r, name="wt")
                           for _ in range(HT)]
                for kj in range(HT):
                    nc.sync.dma_start(
                        wptiles[kj][:],
                        wp_d[kj * P:(kj + 1) * P, :].bitcast(f32r))
            A = apool.tile([P, S], bf16, name="A")
            sc_chunks = []
            for qc in range(QC):
                Sc = scpsum.tile([P, 512], f32, name="Sc")
                nc.tensor.matmul(
                    Sc[:],
                    KT[po:po + 64, j, ki * P:(ki + 1) * P],
                    QT[po:po + 64, j, qc * 512:(qc + 1) * 512],
                    start=True, stop=True)
                sc_chunks.append(Sc)
            if t - LAG >= 0:
                issue_cx(t - LAG, t)
            fill(2)
            # last pair has no projection filler left: keep the PE warm with
            # partial epilogue chunks over the six long-finished head pairs
            if t >= 112 and (t - 112) % 2 == 0:
                qi = (t - 112) // 2
                ps = scpsum.tile([P, 512], f32, name="Sc")
                for m, hj in enumerate(PAIRS[:6]):
                    nc.tensor.matmul(
                        ps[:],
                        r(ctxT[:, hj, qi * P:(qi + 1) * P]),
                        r(wptiles[hj][:, 0:512]),
                        start=(m == 0), stop=(m == 5))
                po2 = o2pool.tile([P, 512], f32, name="po2")
                nc.vector.tensor_copy(po2[:], ps[:])
                partials[(qi, 0)] = po2
            for qc in range(QC):
                nc.scalar.activation(
                    A[:, qc * 512:(qc + 1) * 512], sc_chunks[qc][:],
                    Act.Exp, bias=0.0, scale=float(1.0 / 32.0))
            nc.vector.tensor_tensor(A[:], A[:], E[:, ki, :], op=Alu.mult)
            pend[t] = (A, CX)
            drain_tails(t)
        for t in range(NH * ST - LAG, NH * ST):
            issue_cx(t, NH * ST)
        while tail_q:
            tail_q.pop(0)[1]()

        dpool.release()
        fillps.release()
        cxpsum.release()
        scpsum.release()

        # ---- epilogue: output projection ----
        prpsum = tc.alloc_tile_pool(name="prpsum", bufs=4, space="PSUM")
        opool = tc.alloc_tile_pool(name="opool", bufs=3, side="right")
        for qi in range(ST):
            for dc in range(QC):
                ps = prpsum.tile([P, 512], f32, name="pps")
                hjs = PAIRS[6:] if (qi, dc) in partials else PAIRS
                for n, hj in enumerate(hjs):
                    nc.tensor.matmul(
                        ps[:],
                        r(ctxT[:, hj, qi * P:(qi + 1) * P]),
                        r(wptiles[hj][:, dc * 512:(dc + 1) * 512]),
                        start=(n == 0),
                        stop=(n == len(hjs) - 1 and not with_bias))
                if with_bias:
                    nc.tensor.matmul(
                        ps[:],
                        r(ones_row[:, 0:P]),
                        r(bias_sb[3:4, dc * 512:(dc + 1) * 512]),
                        start=False, stop=True)
                osb = opool.tile([P, 512], f32, name="osb")
                if (qi, dc) in partials:
                    nc.vector.tensor_tensor(
                        osb[:], ps[:], partials[(qi, dc)][:], op=Alu.add)
                else:
                    nc.vector.tensor_copy(osb[:], ps[:])
                nc.sync.dma_start(
                    out_d[qi * P:(qi + 1) * P, dc * 512:(dc + 1) * 512], osb[:])
        opool.release()
        prpsum.release()
        o2pool.release()
        rbp.release()
        rhp.release()
        dnp.release()
        stgp.release()
        apool.release()
        ctxp.release()
        wpool.release()
        ep.release()
        vpp.release()
        ktp.release()
        qtp.release()
        const.release()

    nc.compile()
    return nc


def _get(with_bias):
    if with_bias not in _BUILT:
        _BUILT[with_bias] = _build(with_bias)
    return _BUILT[with_bias]


def _make_in_maps(inputs, with_bias):
    f = lambda a: np.ascontiguousarray(np.asarray(a), dtype=np.float32)
    x = f(inputs["x"])
    mask = f(inputs["attention_mask"])
    shared = {
        "wq": f(inputs["Wq"]), "wk": f(inputs["Wk"]),
        "wv": f(inputs["Wv"]), "wp": f(inputs["Wp"]),
    }
    if with_bias:
        shared["bq"] = f(inputs["bq"]).reshape(1, HID)
        shared["bk"] = f(inputs["bk"]).reshape(1, HID)
        shared["bv"] = f(inputs["bv"]).reshape(1, HID)
        shared["bp"] = f(inputs["bp"]).reshape(1, HID)
    return [
        dict(shared, x=x[b], mask=np.ascontiguousarray(mask[b, 0]))
        for b in range(N_CORES)
    ]


def run(trace=False, **inputs):
    from concourse.bass_utils import run_bass_kernel_spmd
    with_bias = any(
        float(np.abs(np.asarray(inputs[k])).max()) != 0.0
        for k in ("bq", "bk", "bv", "bp"))
    nc = _get(with_bias)
    in_maps = _make_in_maps(inputs, with_bias)
    res = run_bass_kernel_spmd(nc, in_maps, list(range(N_CORES)), trace=trace)
    out = np.stack([res.results[i]["out"] for i in range(N_CORES)])
    return out.astype(np.float32, copy=False), res


def kernel(**inputs):
    out, _ = run(trace=False, **inputs)
    return out
